# revision 11
# baseline (speedup 1.0000x reference)
"""Trainium2 Bass kernel for the AttentionLayer problem.

  Z = tanh(w @ x)            [B, D, L]
  A = softmax(u @ Z, axis=L) [B, C, L]
  V = einsum('bfl,bcl->bcf') [B, C, NF]

Sharding: pure data parallel — B == 8 == n_cores, one batch element per
NeuronCore, no collectives.  Each core computes its full V_b = [C, NF].

Per-core dataflow (all matmuls bf16 with fp32 PSUM accumulation):
  1. Z = tanh(wT.T @ x)                 [D, L]    (PE + ScalarE)
  2. for each class chunk c (512 wide):
       S^T = Z.T @ uT[:, c]             [L, cw]   (PE; Z tiles stationary)
       E^T = exp(S^T)                   [L, cw]   (ScalarE, PSUM -> SBUF bf16)
       V_c = E^T.T @ xT  (+ den = E^T.T @ 1)      (PE; E^T tiles stationary,
                                                   paired N=1 matmul reuses
                                                   the loaded weights)
       out_c = V_c / den                          (DVE reciprocal + scale)
  softmax needs no max subtraction: |S| < ~1 by construction
  (u, w are xavier-scaled, Z = tanh in (-1,1)), so exp never overflows
  and softmax is shift-invariant.
"""

import numpy as np
import ml_dtypes

B, NF, L = 8, 512, 2500
D, C = 256, 8921
C_PAD = 8960  # 70 * 128: every V tile/store is a full 128-partition tile

N_CORES = 8
CCHUNK = 512  # class chunk width (PSUM bank = 512 fp32)

_compiled_nc = None


def _l_tiles():
    """Sequence-dim partition tiles: 2500 = 19*128 + 68."""
    tiles = []
    off = 0
    while off < L:
        tiles.append((off, min(128, L - off)))
        off += 128
    return tiles


def _c_chunks():
    chunks = []
    off = 0
    while off < C:
        chunks.append((off, min(CCHUNK, C - off)))
        off += CCHUNK
    return chunks


def _build_nc():
    import concourse.bass as bass  # noqa: F401
    import concourse.mybir as mybir
    from concourse import bacc
    from concourse.tile import TileContext

    f32 = mybir.dt.float32
    bf16 = mybir.dt.bfloat16
    Tanh = mybir.ActivationFunctionType.Tanh
    Exp = mybir.ActivationFunctionType.Exp

    nc = bacc.Bacc("TRN2", target_bir_lowering=False, debug=False,
                   num_devices=N_CORES)

    xin = nc.declare_dram_parameter("xin", [NF, L], bf16, isOutput=False)
    xT = nc.declare_dram_parameter("xT", [L, NF], bf16, isOutput=False)
    wT = nc.declare_dram_parameter("wT", [NF, D], bf16, isOutput=False)
    uT = nc.declare_dram_parameter("uT", [D, C], bf16, isOutput=False)
    out = nc.declare_dram_parameter("out", [C_PAD, NF], f32, isOutput=True)

    ltiles = _l_tiles()
    nlt = len(ltiles)
    cchunks = _c_chunks()

    with TileContext(nc) as tc:
        with (
            tc.tile_pool(name="consts", bufs=1) as consts,
            tc.tile_pool(name="epool", bufs=2) as epool,
            tc.tile_pool(name="vpool", bufs=4) as vpool,
            tc.tile_pool(name="rpool", bufs=4) as rpool,
            tc.tile_pool(name="spsum", bufs=3, space="PSUM") as spsum,
            tc.tile_pool(name="vpsum", bufs=3, space="PSUM") as vpsum,
            tc.tile_pool(name="dpsum", bufs=2, space="PSUM") as dpsum,
        ):
            # ---- load inputs ----
            # [p, a, ...] layouts: element [p, a, k] = src[a*128 + p, k]
            LCH = 500  # 5 chunks along L for the Z phase
            wTs = consts.tile([128, NF // 128, D], bf16)
            nc.sync.dma_start(out=wTs,
                              in_=wT.ap().rearrange("(a p) d -> p a d", p=128))
            # x split by L-chunk so the Z matmuls start as soon as chunk 0 lands
            xs = consts.tile([128, NF // 128, L], bf16)
            xin_r = xin.ap().rearrange("(a p) l -> p a l", p=128)
            for lc in range(L // LCH):
                nc.sync.dma_start(out=xs[:, :, lc * LCH:(lc + 1) * LCH],
                                  in_=xin_r[:, :, lc * LCH:(lc + 1) * LCH])
            # uT split by class chunk so S of chunk 0 starts early
            uTs = consts.tile([128, D // 128, C], bf16)
            uT_r = uT.ap().rearrange("(a p) c -> p a c", p=128)
            for c0, cw in cchunks:
                nc.sync.dma_start(out=uTs[:, :, c0:c0 + cw],
                                  in_=uT_r[:, :, c0:c0 + cw])
            xTs = consts.tile([128, nlt, NF], bf16)
            nfull = L // 128  # 19 full tiles
            nc.sync.dma_start(
                out=xTs[:, 0:nfull, :],
                in_=xT.ap()[0:nfull * 128, :].rearrange("(a p) f -> p a f", p=128))
            lrem = L - nfull * 128
            nc.sync.dma_start(out=xTs[:lrem, nfull, :],
                              in_=xT.ap()[nfull * 128:L, :])
            ones = consts.tile([128, 1], bf16)
            nc.vector.memset(ones, 1.0)

            # PE warmup: the HAM clock gate holds PE at 1.2 GHz until it has
            # seen ~3.4us of sustained activity.  Burn dummy matmuls while the
            # input DMAs are still in flight so the real work starts at 2.4.
            scratch = consts.tile([128, 512], bf16)
            nc.vector.memset(scratch, 0.03)
            for i in range(36):
                wp = spsum.tile([128, 512], f32, tag="ps")
                nc.tensor.matmul(wp[0:1, :], lhsT=scratch[:, 0:1], rhs=scratch,
                                 start=True, stop=True)

            # ---- Z = tanh(wT.T @ x) : [D, L] as [128, 2, L] bf16 ----
            Zs = consts.tile([128, D // 128, L], bf16)
            for lc in range(L // LCH):
                for dt in range(D // 128):
                    pz = spsum.tile([128, LCH], f32, tag="ps")
                    for f in range(NF // 128):
                        nc.tensor.matmul(
                            pz,
                            lhsT=wTs[:, f, dt * 128:(dt + 1) * 128],
                            rhs=xs[:, f, lc * LCH:(lc + 1) * LCH],
                            start=(f == 0), stop=(f == NF // 128 - 1))
                    nc.scalar.activation(
                        out=Zs[:, dt, lc * LCH:(lc + 1) * LCH], in_=pz, func=Tanh)

            # ---- main loop over class chunks ----
            for c0, cw in cchunks:
                # S^T = Z.T @ uT[:, c0:c0+cw]; E^T = exp(S^T)
                ET = epool.tile([128, nlt, CCHUNK], bf16)
                for lt, (l0, lp) in enumerate(ltiles):
                    ps = spsum.tile([128, CCHUNK], f32)
                    for dt in range(D // 128):
                        nc.tensor.matmul(
                            ps[:lp, :cw],
                            lhsT=Zs[:, dt, l0:l0 + lp],
                            rhs=uTs[:, dt, c0:c0 + cw],
                            start=(dt == 0), stop=(dt == D // 128 - 1))
                    nc.scalar.activation(out=ET[:lp, lt, :cw],
                                         in_=ps[:lp, :cw], func=Exp)

                # V_c = E^T.T @ xT (+ denominator), normalize, store.
                # Always full 128-row tiles: the last chunk's pad rows read
                # stale-but-finite E^T columns and land in out's pad region
                # (C..C_PAD), which the host discards.  Odd-partition DMA
                # stores don't fan out across engines, so this also keeps
                # every store at full DMA bandwidth.
                ncsub = (min(c0 + CCHUNK, C_PAD) - c0 + 127) // 128
                for csi in range(ncsub):
                    cs0 = csi * 128
                    pv = vpsum.tile([128, NF], f32)
                    pd = dpsum.tile([128, 1], f32)
                    for lt, (l0, lp) in enumerate(ltiles):
                        nc.tensor.matmul(
                            pv,
                            lhsT=ET[:lp, lt, cs0:cs0 + 128],
                            rhs=xTs[:lp, lt, :],
                            start=(lt == 0), stop=(lt == nlt - 1))
                        nc.tensor.matmul(
                            pd,
                            lhsT=ET[:lp, lt, cs0:cs0 + 128],
                            rhs=ones[:lp, :],
                            start=(lt == 0), stop=(lt == nlt - 1))
                    rr = rpool.tile([128, 1], f32)
                    nc.vector.reciprocal(rr, pd)
                    vo = vpool.tile([128, NF], f32)
                    nc.vector.tensor_scalar_mul(vo, pv, rr)
                    nc.sync.dma_start(out=out.ap()[c0 + cs0:c0 + cs0 + 128, :],
                                      in_=vo)

    nc.compile()
    return nc


def _get_nc():
    global _compiled_nc
    if _compiled_nc is None:
        _compiled_nc = _build_nc()
    return _compiled_nc


def _make_in_maps(x, w, u):
    bf = ml_dtypes.bfloat16
    wT = np.ascontiguousarray(np.asarray(w, np.float32).T).astype(bf)
    uT = np.ascontiguousarray(np.asarray(u, np.float32).T).astype(bf)
    in_maps = []
    for i in range(N_CORES):
        xb = np.asarray(x[i], np.float32)
        in_maps.append({
            "xin": xb.astype(bf),
            "xT": np.ascontiguousarray(xb.T).astype(bf),
            "wT": wT,
            "uT": uT,
        })
    return in_maps


def _run(x, w, u, trace=False):
    from concourse.bass_utils import run_bass_kernel_spmd

    nc = _get_nc()
    in_maps = _make_in_maps(x, w, u)
    res = run_bass_kernel_spmd(nc, in_maps, list(range(N_CORES)), trace=trace)
    outv = np.stack([res.results[i]["out"][:C] for i in range(N_CORES)], axis=0)
    return outv.astype(np.float32), res


def kernel(x, w, u):
    outv, _ = _run(x, w, u, trace=False)
    return outv


# revision 12
# speedup vs baseline: 1.2044x; 1.2044x over previous
"""Trainium2 Bass kernel for the AttentionLayer problem.

  Z = tanh(w @ x)            [B, D, L]
  A = softmax(u @ Z, axis=L) [B, C, L]
  V = einsum('bfl,bcl->bcf') [B, C, NF]

Sharding: pure data parallel — B == 8 == n_cores, one batch element per
NeuronCore, no collectives.  Each core computes its full V_b = [C, NF].

Per-core dataflow (all matmuls bf16 with fp32 PSUM accumulation):
  1. Z = tanh(wT.T @ x)                 [D, L]    (PE + ScalarE)
  2. for each class chunk c (512 wide):
       S^T = Z.T @ uT[:, c]             [L, cw]   (PE; Z tiles stationary)
       E^T = exp(S^T)                   [L, cw]   (ScalarE, PSUM -> SBUF bf16)
       V_c = E^T.T @ xT  (+ den = E^T.T @ 1)      (PE; E^T tiles stationary,
                                                   paired N=1 matmul reuses
                                                   the loaded weights)
       out_c = V_c / den                          (DVE reciprocal + scale)
  softmax needs no max subtraction: |S| < ~1 by construction
  (u, w are xavier-scaled, Z = tanh in (-1,1)), so exp never overflows
  and softmax is shift-invariant.
"""

import numpy as np
import ml_dtypes

B, NF, L = 8, 512, 2500
D, C = 256, 8921
C_PAD = 8960  # 70 * 128: every V tile/store is a full 128-partition tile

N_CORES = 8
CCHUNK = 512  # class chunk width (PSUM bank = 512 fp32)

_compiled_nc = None


def _l_tiles():
    """Sequence-dim partition tiles: 2500 = 19*128 + 68."""
    tiles = []
    off = 0
    while off < L:
        tiles.append((off, min(128, L - off)))
        off += 128
    return tiles


def _c_chunks():
    chunks = []
    off = 0
    while off < C:
        chunks.append((off, min(CCHUNK, C - off)))
        off += CCHUNK
    return chunks


def _build_nc():
    import concourse.bass as bass  # noqa: F401
    import concourse.mybir as mybir
    from concourse import bacc
    from concourse.tile import TileContext

    f32 = mybir.dt.float32
    bf16 = mybir.dt.bfloat16
    Tanh = mybir.ActivationFunctionType.Tanh
    Exp = mybir.ActivationFunctionType.Exp

    nc = bacc.Bacc("TRN2", target_bir_lowering=False, debug=False,
                   num_devices=N_CORES)

    xin = nc.declare_dram_parameter("xin", [NF, L], bf16, isOutput=False)
    xT = nc.declare_dram_parameter("xT", [L, NF], bf16, isOutput=False)
    wT = nc.declare_dram_parameter("wT", [NF, D], bf16, isOutput=False)
    uT = nc.declare_dram_parameter("uT", [D, C], bf16, isOutput=False)
    out = nc.declare_dram_parameter("out", [C_PAD, NF], f32, isOutput=True)

    ltiles = _l_tiles()
    nlt = len(ltiles)
    cchunks = _c_chunks()

    with TileContext(nc) as tc:
        with (
            tc.tile_pool(name="consts", bufs=1) as consts,
            tc.tile_pool(name="epool", bufs=2) as epool,
            tc.tile_pool(name="vpool", bufs=4) as vpool,
            tc.tile_pool(name="rpool", bufs=4) as rpool,
            tc.tile_pool(name="spsum", bufs=3, space="PSUM") as spsum,
            tc.tile_pool(name="vpsum", bufs=3, space="PSUM") as vpsum,
            tc.tile_pool(name="dpsum", bufs=2, space="PSUM") as dpsum,
        ):
            # ---- load inputs ----
            # [p, a, ...] layouts: element [p, a, k] = src[a*128 + p, k]
            LCH = 500  # 5 chunks along L for the Z phase
            wTs = consts.tile([128, NF // 128, D], bf16)
            nc.sync.dma_start(out=wTs,
                              in_=wT.ap().rearrange("(a p) d -> p a d", p=128))
            # x split by L-chunk so the Z matmuls start as soon as chunk 0 lands
            xs = consts.tile([128, NF // 128, L], bf16)
            xin_r = xin.ap().rearrange("(a p) l -> p a l", p=128)
            for lc in range(L // LCH):
                nc.sync.dma_start(out=xs[:, :, lc * LCH:(lc + 1) * LCH],
                                  in_=xin_r[:, :, lc * LCH:(lc + 1) * LCH])
            # uT split by class chunk so S of chunk 0 starts early
            uTs = consts.tile([128, D // 128, C], bf16)
            uT_r = uT.ap().rearrange("(a p) c -> p a c", p=128)
            for c0, cw in cchunks:
                nc.sync.dma_start(out=uTs[:, :, c0:c0 + cw],
                                  in_=uT_r[:, :, c0:c0 + cw])
            xTs = consts.tile([128, nlt, NF], bf16)
            nfull = L // 128  # 19 full tiles
            nc.sync.dma_start(
                out=xTs[:, 0:nfull, :],
                in_=xT.ap()[0:nfull * 128, :].rearrange("(a p) f -> p a f", p=128))
            lrem = L - nfull * 128
            nc.sync.dma_start(out=xTs[:lrem, nfull, :],
                              in_=xT.ap()[nfull * 128:L, :])
            ones = consts.tile([128, 1], bf16)
            nc.vector.memset(ones, 1.0)



            # ---- Z = tanh(wT.T @ x) : [D, L] as [128, 2, L] bf16 ----
            Zs = consts.tile([128, D // 128, L], bf16)
            for lc in range(L // LCH):
                for dt in range(D // 128):
                    pz = spsum.tile([128, LCH], f32, tag="ps")
                    for f in range(NF // 128):
                        nc.tensor.matmul(
                            pz,
                            lhsT=wTs[:, f, dt * 128:(dt + 1) * 128],
                            rhs=xs[:, f, lc * LCH:(lc + 1) * LCH],
                            start=(f == 0), stop=(f == NF // 128 - 1))
                    nc.scalar.activation(
                        out=Zs[:, dt, lc * LCH:(lc + 1) * LCH], in_=pz, func=Tanh)

            # ---- main loop over class chunks ----
            for c0, cw in cchunks:
                # S^T = Z.T @ uT[:, c0:c0+cw]; E^T = exp(S^T)
                ET = epool.tile([128, nlt, CCHUNK], bf16)
                for lt, (l0, lp) in enumerate(ltiles):
                    ps = spsum.tile([128, CCHUNK], f32)
                    for dt in range(D // 128):
                        nc.tensor.matmul(
                            ps[:lp, :cw],
                            lhsT=Zs[:, dt, l0:l0 + lp],
                            rhs=uTs[:, dt, c0:c0 + cw],
                            start=(dt == 0), stop=(dt == D // 128 - 1))
                    nc.scalar.activation(out=ET[:lp, lt, :cw],
                                         in_=ps[:lp, :cw], func=Exp)

                # V_c = E^T.T @ xT (+ denominator), normalize, store.
                # Always full 128-row tiles: the last chunk's pad rows read
                # stale-but-finite E^T columns and land in out's pad region
                # (C..C_PAD), which the host discards.  Odd-partition DMA
                # stores don't fan out across engines, so this also keeps
                # every store at full DMA bandwidth.
                ncsub = (min(c0 + CCHUNK, C_PAD) - c0 + 127) // 128
                for csi in range(ncsub):
                    cs0 = csi * 128
                    pv = vpsum.tile([128, NF], f32)
                    pd = dpsum.tile([128, 1], f32)
                    for lt, (l0, lp) in enumerate(ltiles):
                        nc.tensor.matmul(
                            pv,
                            lhsT=ET[:lp, lt, cs0:cs0 + 128],
                            rhs=xTs[:lp, lt, :],
                            start=(lt == 0), stop=(lt == nlt - 1))
                        nc.tensor.matmul(
                            pd,
                            lhsT=ET[:lp, lt, cs0:cs0 + 128],
                            rhs=ones[:lp, :],
                            start=(lt == 0), stop=(lt == nlt - 1))
                    rr = rpool.tile([128, 1], f32)
                    nc.vector.reciprocal(rr, pd)
                    vo = vpool.tile([128, NF], f32)
                    nc.vector.tensor_scalar_mul(vo, pv, rr)
                    nc.sync.dma_start(out=out.ap()[c0 + cs0:c0 + cs0 + 128, :],
                                      in_=vo)

    nc.compile()
    return nc


def _get_nc():
    global _compiled_nc
    if _compiled_nc is None:
        _compiled_nc = _build_nc()
    return _compiled_nc


def _make_in_maps(x, w, u):
    bf = ml_dtypes.bfloat16
    wT = np.ascontiguousarray(np.asarray(w, np.float32).T).astype(bf)
    uT = np.ascontiguousarray(np.asarray(u, np.float32).T).astype(bf)
    in_maps = []
    for i in range(N_CORES):
        xb = np.asarray(x[i], np.float32)
        in_maps.append({
            "xin": xb.astype(bf),
            "xT": np.ascontiguousarray(xb.T).astype(bf),
            "wT": wT,
            "uT": uT,
        })
    return in_maps


def _run(x, w, u, trace=False):
    from concourse.bass_utils import run_bass_kernel_spmd

    nc = _get_nc()
    in_maps = _make_in_maps(x, w, u)
    res = run_bass_kernel_spmd(nc, in_maps, list(range(N_CORES)), trace=trace)
    outv = np.stack([res.results[i]["out"][:C] for i in range(N_CORES)], axis=0)
    return outv.astype(np.float32), res


def kernel(x, w, u):
    outv, _ = _run(x, w, u, trace=False)
    return outv
